# revision 4
# baseline (speedup 1.0000x reference)
"""Trainium2 kernel for AxialTranslatorRotator.translate (translate=True path).

The reference scatters the input block x[b] (C=32, H=382, W=255) into a
zero-initialized (C, 765, 765) output at a per-batch offset:

    out[b, c, j + u0_b, i + v0_b] = x[b, c, j, i]
    u0_b = 255 - (q_b - r_b // 2),  v0_b = 382 - r_b

Sharding: data-parallel over batch B=16 -> 2 batches per core on 8 cores.

Implementation notes:
  * Dynamic-offset DMAs lower to a per-descriptor ucode patch chain on this
    backend (IndirectSave), which is orders of magnitude slower than static
    descriptors, so the per-batch placements are baked into the program as
    static DRAM->DRAM DMA access patterns. Each core picks its own pair of
    block copies via a branch on the partition id (one SPMD program). The
    program is compiled per distinct offset tuple and cached.
  * Raw Bacc (no TileContext): a pure-DMA Tile kernel accumulates one tail
    drain wait per DMAHW sem lane, which the TPB_CTRL codegen rejects
    ("Too many sync wait commands"). With raw Bacc we use one explicit
    semaphore and a single wait.
  * Output DRAM buffers are pre-zeroed by run_bass_kernel_spmd (documented
    contract: "kernels that don't write every element rely on that"), so
    only the scattered blocks are written.
"""

import numpy as np

import concourse.bacc as bacc
import concourse.mybir as mybir
from concourse.bass_utils import run_bass_kernel_spmd

B, C, H, W = 16, 32, 382, 255  # full input shape
ADD = H - W                    # 127
S = 2 * H + 1                  # 765
N_CORES = 8
B_LOC = B // N_CORES           # 2 batches per core

# Channels per DMA (several DMAs in flight per batch).
CH_CHUNK = 16
DMAS_PER_CORE = B_LOC * (C // CH_CHUNK)

_CACHE: dict = {}


def _build_program(u0v0: tuple):
    """u0v0: tuple of 16 (u0, v0) placements, one per global batch."""
    nc = bacc.Bacc()
    x = nc.dram_tensor(
        "x", [B_LOC, C, H, W], mybir.dt.float32, kind="ExternalInput"
    )
    out = nc.dram_tensor(
        "out", [B_LOC, C, S, S], mybir.dt.float32, kind="ExternalOutput"
    )

    with nc.Block() as block, nc.semaphore() as dsem:

        @block.sync
        def _(sync):
            pid = sync.partition_id()
            for core in range(N_CORES):
                with sync.If(pid == core):
                    for b in range(B_LOC):
                        u0, v0 = u0v0[core * B_LOC + b]
                        u0, v0 = int(u0), int(v0)
                        for c0 in range(0, C, CH_CHUNK):
                            sync.dma_start(
                                out[b, c0 : c0 + CH_CHUNK, u0 : u0 + H, v0 : v0 + W],
                                x[b, c0 : c0 + CH_CHUNK],
                            ).then_inc(dsem, 16)
            sync.wait_ge(dsem, 16 * DMAS_PER_CORE)

    nc.compile()
    return nc


def _get_program(u0v0: tuple):
    if u0v0 not in _CACHE:
        _CACHE[u0v0] = _build_program(u0v0)
    return _CACHE[u0v0]


def _placements(offset: np.ndarray):
    q = offset[:, 0]
    r = offset[:, 1]
    uoff = q - r // 2
    u_min = (H - ADD) - uoff
    v_min = H - r
    return u_min, v_min


def _run(axial_tensor: np.ndarray, offset: np.ndarray, **run_kwargs):
    axial_tensor = np.ascontiguousarray(axial_tensor, dtype=np.float32)
    offset = np.asarray(offset)
    u_min, v_min = _placements(offset)
    u0v0 = tuple((int(u_min[i]), int(v_min[i])) for i in range(B))
    nc = _get_program(u0v0)
    in_maps = [
        {"x": axial_tensor[core * B_LOC : (core + 1) * B_LOC]}
        for core in range(N_CORES)
    ]
    res = run_bass_kernel_spmd(
        nc, in_maps, core_ids=list(range(N_CORES)), **run_kwargs
    )
    out = np.concatenate([m["out"] for m in res.results], axis=0)
    return out, u_min, v_min, res


def kernel(axial_tensor: np.ndarray, offset: np.ndarray):
    out, u_min, v_min, _ = _run(axial_tensor, offset)
    u_max = u_min + H
    v_max = v_min + W
    return out, u_min, u_max, v_min, v_max


def profile(axial_tensor: np.ndarray, offset: np.ndarray):
    """Run once with NTFF tracing; returns (exec_time_ns, BassKernelResults)."""
    _, _, _, res = _run(axial_tensor, offset, trace=True)
    return res.exec_time_ns, res


# revision 5
# speedup vs baseline: 1.0628x; 1.0628x over previous
"""Trainium2 kernel for AxialTranslatorRotator.translate (translate=True path).

The reference scatters the input block x[b] (C=32, H=382, W=255) into a
zero-initialized (C, 765, 765) output at a per-batch offset:

    out[b, c, j + u0_b, i + v0_b] = x[b, c, j, i]
    u0_b = 255 - (q_b - r_b // 2),  v0_b = 382 - r_b

Sharding: data-parallel over batch B=16 -> 2 batches per core on 8 cores.

Implementation notes:
  * Dynamic-offset DMAs lower to a per-descriptor ucode patch chain on this
    backend (IndirectSave), which is orders of magnitude slower than static
    descriptors, so the per-batch placements are baked into the program as
    static DRAM->DRAM DMA access patterns. Each core picks its own pair of
    block copies via a branch on the partition id (one SPMD program). The
    program is compiled per distinct offset tuple and cached.
  * Raw Bacc (no TileContext): a pure-DMA Tile kernel accumulates one tail
    drain wait per DMAHW sem lane, which the TPB_CTRL codegen rejects
    ("Too many sync wait commands"). With raw Bacc we use one explicit
    semaphore and a single wait.
  * Output DRAM buffers are pre-zeroed by run_bass_kernel_spmd (documented
    contract: "kernels that don't write every element rely on that"), so
    only the scattered blocks are written.
"""

import numpy as np

import concourse.bacc as bacc
import concourse.mybir as mybir
from concourse.bass_utils import run_bass_kernel_spmd

B, C, H, W = 16, 32, 382, 255  # full input shape
ADD = H - W                    # 127
S = 2 * H + 1                  # 765
N_CORES = 8
B_LOC = B // N_CORES           # 2 batches per core

# Channels per DMA (several DMAs in flight per batch).
CH_CHUNK = 16
DMAS_PER_CORE = B_LOC * (C // CH_CHUNK)

_CACHE: dict = {}


def _build_program(u0v0: tuple):
    """u0v0: tuple of 16 (u0, v0) placements, one per global batch."""
    nc = bacc.Bacc()
    x = nc.dram_tensor(
        "x", [B_LOC, C, H, W], mybir.dt.float32, kind="ExternalInput"
    )
    out = nc.dram_tensor(
        "out", [B_LOC, C, S, S], mybir.dt.float32, kind="ExternalOutput"
    )

    with nc.Block() as block, nc.semaphore() as dsem:

        @block.sync
        def _(sync):
            pid = sync.partition_id()
            for core in range(N_CORES):
                with sync.If(pid == core):
                    for b in range(B_LOC):
                        u0, v0 = u0v0[core * B_LOC + b]
                        u0, v0 = int(u0), int(v0)
                        for c0 in range(0, C, CH_CHUNK):
                            sync.dma_start(
                                out[b, c0 : c0 + CH_CHUNK, u0 : u0 + H, v0 : v0 + W],
                                x[b, c0 : c0 + CH_CHUNK],
                            ).then_inc(dsem, 16)
            sync.wait_ge(dsem, 16 * DMAS_PER_CORE)

    nc.compile()
    return nc


def _get_program(u0v0: tuple):
    if u0v0 not in _CACHE:
        _CACHE[u0v0] = _build_program(u0v0)
    return _CACHE[u0v0]


def _placements(offset: np.ndarray):
    q = offset[:, 0]
    r = offset[:, 1]
    uoff = q - r // 2
    u_min = (H - ADD) - uoff
    v_min = H - r
    return u_min, v_min


def _run(axial_tensor: np.ndarray, offset: np.ndarray, **run_kwargs):
    axial_tensor = np.ascontiguousarray(axial_tensor, dtype=np.float32)
    offset = np.asarray(offset)
    u_min, v_min = _placements(offset)
    u0v0 = tuple((int(u_min[i]), int(v_min[i])) for i in range(B))
    nc = _get_program(u0v0)
    in_maps = [
        {"x": axial_tensor[core * B_LOC : (core + 1) * B_LOC]}
        for core in range(N_CORES)
    ]
    try:
        res = run_bass_kernel_spmd(
            nc, in_maps, core_ids=list(range(N_CORES)), **run_kwargs
        )
    except Exception:
        # Transient device errors (e.g. NRT_EXEC_UNIT_UNRECOVERABLE) have
        # been observed once after heavy benchmarking; retry once.
        res = run_bass_kernel_spmd(
            nc, in_maps, core_ids=list(range(N_CORES)), **run_kwargs
        )
    out = np.concatenate([m["out"] for m in res.results], axis=0)
    return out, u_min, v_min, res


def kernel(axial_tensor: np.ndarray, offset: np.ndarray):
    out, u_min, v_min, _ = _run(axial_tensor, offset)
    u_max = u_min + H
    v_max = v_min + W
    return out, u_min, u_max, v_min, v_max


def profile(axial_tensor: np.ndarray, offset: np.ndarray):
    """Run once with NTFF tracing; returns (exec_time_ns, BassKernelResults)."""
    _, _, _, res = _run(axial_tensor, offset, trace=True)
    return res.exec_time_ns, res
